# revision 25
# baseline (speedup 1.0000x reference)
"""Trainium2 Bass kernel for nn_Agent_12489764897159 (moe_routing actor-critic MLP).

Data-parallel over 8 NeuronCores: batch B=131072 split into 8 x 16384.
All weights replicated (fused/padded host-side into 3 matmul operands).

Per-core device pipeline (feature-major trunk, batch-major postprocessing):
  x [16384,194] --DMA--> SBUF batch-major --PE transpose--> xT feature-major
  out1 = W1cat.T @ xT          (fp32r matmuls, actor+critic trunks fused: 64+64 rows)
  h1   = tanh(out1 + b1cat)    (ACT, PSUM->SBUF)
  out2 = W2cat.T @ h1; h2 = tanh(out2 + b2cat)
  o3   = W3cat.T @ h2          (col-tiled 4x32: [all_logits(15) | value | pad] stacked x4)
  o3s  = o3 + bias3            (DVE, PSUM->SBUF)
  o3bm = PE-transpose(o3s)     (batch-major: samples on partitions)
  POST accumulates o3bm; postprocessing (event-mask select, softmax stats,
  entropy, logp gather by action) runs once per core on [128, 4096] views.
"""

import numpy as np
import concourse.bacc as bacc
import concourse.bass as bass
import concourse.mybir as mybir
from concourse import tile
from concourse.bass_utils import run_bass_kernel_spmd

F32 = mybir.dt.float32
F32R = mybir.dt.float32r
BF16 = mybir.dt.bfloat16
I32 = mybir.dt.int32
AF = mybir.ActivationFunctionType
OP = mybir.AluOpType

NCORES = 8
B, OBS, H, A, E = 131072, 194, 64, 5, 3
BC = B // NCORES          # 16384 samples per core
NCH = 8                   # chunks per core
CH = 2048                 # samples per chunk
G_PER_CHUNK = 16          # 128-sample groups per chunk
NG = NCH * G_PER_CHUNK    # 128 groups per core

_CACHE = {}
LAST_RESULT = None


def _ap(t, offset, dims):
    return bass.AP(t.tensor if isinstance(t, bass.AP) else t, offset, dims)


def _build_program(nc):
    xg = nc.dram_tensor("xg", [BC, OBS], F32, kind="ExternalInput")
    act = nc.dram_tensor("act", [BC], F32, kind="ExternalInput")
    w1 = nc.dram_tensor("w1cat", [128, 128], BF16, kind="ExternalInput")
    w1b = nc.dram_tensor("w1catb", [128, 128], BF16, kind="ExternalInput")
    w2 = nc.dram_tensor("w2cat", [128, 128], F32R, kind="ExternalInput")
    w3 = nc.dram_tensor("w3cat", [128, 32], BF16, kind="ExternalInput")
    b1 = nc.dram_tensor("b1cat", [128, 1], F32, kind="ExternalInput")
    b2 = nc.dram_tensor("b2cat", [128, 1], F32, kind="ExternalInput")
    b512 = nc.dram_tensor("b512", [128, 512], F32, kind="ExternalInput")
    idm = nc.dram_tensor("idm", [128, 128], F32, kind="ExternalInput")
    iota = nc.dram_tensor("iota", [128, 5], F32, kind="ExternalInput")
    out3_o = nc.dram_tensor("out3_o", [3, BC], F32, kind="ExternalOutput")

    with tile.TileContext(nc) as tc:
        with (
            tc.tile_pool(name="constp", bufs=1) as constp,
            tc.tile_pool(name="xrp", bufs=3) as xrp,
            tc.tile_pool(name="xbmp", bufs=3) as xbmp,
            tc.tile_pool(name="xtsp", bufs=3) as xtsp,
            tc.tile_pool(name="hp", bufs=3) as hp,
            tc.tile_pool(name="bigp", bufs=1) as bigp,
            tc.tile_pool(name="tpp", bufs=1, space="PSUM") as tpp,
            tc.tile_pool(name="mmp", bufs=5, space="PSUM") as mmp,
            tc.tile_pool(name="o3bmp", bufs=2, space="PSUM") as o3bmp,
        ):
            # ---- constants ----
            w1a_sb = constp.tile([128, 128], BF16)
            nc.scalar.dma_start(out=w1a_sb[:], in_=w1.ap())
            w1b_sb = constp.tile([128, 128], BF16)
            nc.scalar.dma_start(out=w1b_sb[:], in_=w1b.ap())
            w2_sb = constp.tile([128, 128], F32R)
            nc.scalar.dma_start(out=w2_sb[:], in_=w2.ap())
            w3_sb = constp.tile([128, 32], BF16)
            nc.scalar.dma_start(out=w3_sb[:], in_=w3.ap())
            b1_sb = constp.tile([128, 1], F32)
            nc.scalar.dma_start(out=b1_sb[:], in_=b1.ap())
            b2_sb = constp.tile([128, 1], F32)
            nc.scalar.dma_start(out=b2_sb[:], in_=b2.ap())
            b512_sb = constp.tile([128, 512], F32)
            nc.scalar.dma_start(out=b512_sb[:], in_=b512.ap())
            idm_sb = constp.tile([128, 128], F32)
            nc.scalar.dma_start(out=idm_sb[:], in_=idm.ap())
            iota_sb = constp.tile([128, 5], F32)
            nc.scalar.dma_start(out=iota_sb[:], in_=iota.ap())

            # ---- persistent per-core tiles ----
            POST = bigp.tile([128, 32 * NG], F32)     # col = 512c+128t+32j+m
            x012 = bigp.tile([128, 3 * NG], F32)      # col = 3G+i, G=16c+4t+j
            actf = bigp.tile([128, NG], F32)

            # ---- action: dense k-major load, PE transpose to batch-major ----
            # host supplies act as fp32; act[k, p] with sample = 128k + p
            actk = bigp.tile([128, 128], F32)
            nc.scalar.dma_start(out=actk[:], in_=_ap(act, 0, [[128, 128], [1, 128]]))
            actp = tpp.tile([128, 512], F32, tag="tp", name="actp")
            nc.tensor.transpose(actp[:, 0:128], actk[:], idm_sb[:])
            # psum[p, k] -> actf[p, G] with G = 16c+4t+j, k = 16c+4j+t
            nc.vector.tensor_copy(
                _ap(actf, 0, [actf.ap[0], [16, 8], [4, 4], [1, 4]]),
                _ap(actp, actp.offset, [actp.ap[0], [16, 8], [1, 4], [4, 4]]))

            # ---- main per-chunk pipeline (software-pipelined emission) ----
            state = {}

            def emit_input(c):
                # fp32 batch-major load, large descriptors (194-elem runs)
                x_bm = xbmp.tile([128, 16 * OBS], F32)
                nc.gpsimd.dma_start(
                    out=_ap(x_bm, 0, [x_bm.ap[0], [OBS, 16], [1, OBS]]),
                    in_=_ap(xg, c * CH * OBS, [[OBS, 128], [128 * OBS, 16], [1, OBS]]),
                )
                # extract x[:, 0:3] (fp32) for the event masks: blk = 4j + t
                nc.vector.tensor_copy(
                    _ap(x012, 48 * c, [x012.ap[0], [3, 4], [12, 4], [1, 3]]),
                    _ap(x_bm, 0, [x_bm.ap[0], [4 * OBS, 4], [OBS, 4], [1, 3]]))
                # cast+reshuffle f32 -> bf16 xbar layout:
                # R1 cols 0:2048 = (blk, f 0:128); R2 = (blk, 66+g) overlap window
                xr = xrp.tile([128, 4096], BF16)
                nc.vector.tensor_copy(
                    _ap(xr, 0, [xr.ap[0], [128, 16], [1, 128]]),
                    _ap(x_bm, 0, [x_bm.ap[0], [OBS, 16], [1, 128]]))
                nc.vector.tensor_copy(
                    _ap(xr, 2048, [xr.ap[0], [128, 16], [1, 128]]),
                    _ap(x_bm, 66, [x_bm.ap[0], [OBS, 16], [1, 128]]))
                xT = xtsp.tile([128, 2 * CH], BF16, tag="xT")
                nc.sync.dma_start_transpose(
                    xT.rearrange("f (b s) -> f b s", s=128), xr[:])

                state[c] = xT

            def emit_compute(c):
                xT = state.pop(c)
                h1 = hp.tile([128, CH], F32R, tag="h", name=f"h1_{c}")
                h2 = hp.tile([128, CH], BF16, tag="h2", name=f"h2_{c}")
                o12s = [mmp.tile([128, 512], F32, tag="mm", name=f"o1_{c}_{j}")
                        for j in range(4)]
                for j in range(4):
                    nc.tensor.matmul(
                        o12s[j][:], w1a_sb[:],
                        xT[:, 512 * j:512 * j + 512],
                        start=True, stop=False)
                for j in range(4):
                    nc.tensor.matmul(
                        o12s[j][:], w1b_sb[:],
                        xT[:, CH + 512 * j:CH + 512 * j + 512],
                        start=False, stop=True)
                for j in range(4):
                    nc.scalar.activation(
                        out=h1[:, 512 * j:512 * j + 512], in_=o12s[j][:],
                        func=AF.Tanh, bias=b1_sb[:], scale=1.0)
                for j in range(4):
                    o12 = mmp.tile([128, 512], F32, tag="mm", name=f"o2_{c}_{j}")
                    nc.tensor.matmul(
                        o12[:], w2_sb[:],
                        h1[:, 512 * j:512 * j + 512])
                    nc.scalar.activation(
                        out=h2[:, 512 * j:512 * j + 512], in_=o12[:],
                        func=AF.Tanh, bias=b2_sb[:], scale=1.0)

                # batch-stationary head matmuls: lhsT = h2 block ->
                # out [128 samples, 32 metrics] batch-major directly
                o3bm = o3bmp.tile([128, 512], F32, tag="o3bm", name=f"o3bm_{c}")
                for j in range(4):
                    for t in range(4):
                        nc.tensor.matmul(
                            o3bm[:, 32 * (4 * t + j):32 * (4 * t + j) + 32],
                            h2[:, 512 * j + 128 * t:512 * j + 128 * t + 128],
                            w3_sb[:])
                # POST accumulate with bias pattern (bh | bc3) folded in
                nc.vector.tensor_tensor(
                    POST[:, 512 * c:512 * c + 512], o3bm[:], b512_sb[:], op=OP.add)

            LA = 2
            for c in range(NCH + LA):
                if c < NCH:
                    emit_input(c)
                if c >= LA:
                    emit_compute(c - LA)

            # ---- per-core postprocessing (batch-major, G=128 groups) ----
            P3 = POST.rearrange("p (g m) -> p g m", m=32)      # [128,128,32]
            X3 = x012.rearrange("p (g i) -> p g i", i=3)

            ge01 = bigp.tile([128, NG], F32)
            ge02 = bigp.tile([128, NG], F32)
            ge12 = bigp.tile([128, NG], F32)
            nc.vector.tensor_tensor(ge01[:], X3[:, :, 0], X3[:, :, 1], op=OP.is_ge)
            nc.vector.tensor_tensor(ge02[:], X3[:, :, 0], X3[:, :, 2], op=OP.is_ge)
            nc.vector.tensor_tensor(ge12[:], X3[:, :, 1], X3[:, :, 2], op=OP.is_ge)
            m0 = bigp.tile([128, NG], F32)
            nc.vector.tensor_mul(m0[:], ge01[:], ge02[:])
            m1t = bigp.tile([128, NG], F32)
            nc.vector.tensor_mul(m1t[:], ge01[:], ge12[:])
            m1 = bigp.tile([128, NG], F32)
            nc.vector.tensor_sub(m1[:], ge12[:], m1t[:])

            L0 = P3[:, :, 0:5]
            L1 = P3[:, :, 5:10]
            L2 = P3[:, :, 10:15]
            D0 = bigp.tile([128, 5 * NG], F32)
            D1 = bigp.tile([128, 5 * NG], F32)
            D0r = D0.rearrange("p (g a) -> p g a", a=5)
            D1r = D1.rearrange("p (g a) -> p g a", a=5)
            nc.vector.tensor_sub(D0r, L0, L2)
            nc.vector.tensor_sub(D1r, L1, L2)

            def bcast5(m):
                return _ap(m, m.offset, [m.ap[0], m.ap[1], [0, 5]])

            SEL = bigp.tile([128, 5 * NG], F32)
            SELr = SEL.rearrange("p (g a) -> p g a", a=5)
            # SEL = L2 + m0*D0 + m1*D1
            nc.vector.tensor_mul(D0r, D0r, bcast5(m0))
            nc.vector.tensor_mul(D1r, D1r, bcast5(m1))
            nc.vector.tensor_add(SELr, L2, D0r)
            nc.vector.tensor_add(SELr, SELr, D1r)

            OH = bigp.tile([128, 5 * NG], F32)
            OHr = OH.rearrange("p (g a) -> p g a", a=5)
            actb = _ap(actf, actf.offset, [actf.ap[0], actf.ap[1], [0, 5]])
            iotab = _ap(iota_sb, iota_sb.offset, [iota_sb.ap[0], [0, NG], [1, 5]])
            nc.vector.tensor_tensor(OHr, actb, iotab, op=OP.is_equal)
            LSW = bigp.tile([128, 5 * NG], F32)
            LSWr = LSW.rearrange("p (g a) -> p g a", a=5)
            nc.vector.tensor_mul(LSWr, SELr, OHr)
            LSEL = bigp.tile([128, NG], F32)
            nc.vector.tensor_reduce(LSEL[:], LSWr, axis=mybir.AxisListType.X, op=OP.add)

            EXP = bigp.tile([128, 5 * NG], F32)
            EXPr = EXP.rearrange("p (g a) -> p g a", a=5)
            nc.scalar.activation(out=EXP[:], in_=SEL[:], func=AF.Exp)
            S = bigp.tile([128, NG], F32)
            nc.vector.tensor_reduce(S[:], EXPr, axis=mybir.AxisListType.X, op=OP.add)
            TW = bigp.tile([128, 5 * NG], F32)
            TWr = TW.rearrange("p (g a) -> p g a", a=5)
            nc.vector.tensor_mul(TWr, SELr, EXPr)
            T = bigp.tile([128, NG], F32)
            nc.vector.tensor_reduce(T[:], TWr, axis=mybir.AxisListType.X, op=OP.add)
            logS = bigp.tile([128, NG], F32)
            nc.scalar.activation(out=logS[:], in_=S[:], func=AF.Ln)
            Sinv = bigp.tile([128, NG], F32)
            nc.vector.reciprocal(Sinv[:], S[:])
            TSi = bigp.tile([128, NG], F32)
            nc.vector.tensor_mul(TSi[:], T[:], Sinv[:])

            # outputs staged k-ordered (k = 16c + 4j + t) for contiguous stores
            outst = bigp.tile([128, 384], F32)

            def kview(mi):
                # write view enumerating (c, t, j): col = 128*mi + 16c + 4j + t
                return _ap(outst, outst.offset + 128 * mi,
                           [outst.ap[0], [16, 8], [1, 4], [4, 4]])

            def gview(tl):
                return _ap(tl, tl.offset, [tl.ap[0], [16, 8], [4, 4], [1, 4]])

            nc.vector.tensor_tensor(kview(0), gview(LSEL), gview(logS), op=OP.subtract)
            nc.vector.tensor_tensor(kview(1), gview(logS), gview(TSi), op=OP.subtract)
            nc.vector.tensor_tensor(
                kview(2),
                _ap(POST, POST.offset + 15, [POST.ap[0], [512, 8], [128, 4], [32, 4]]),
                _ap(POST, POST.offset + 16, [POST.ap[0], [512, 8], [128, 4], [32, 4]]),
                op=OP.add)

            otp = tpp.tile([128, 512], F32, tag="tp")
            for mi in range(3):
                nc.tensor.transpose(
                    otp[:, 128 * mi:128 * mi + 128],
                    outst[:, 128 * mi:128 * mi + 128], idm_sb[:])
            outsb = bigp.tile([128, 384], F32)
            nc.vector.tensor_copy(outsb[:], otp[:, 0:384])

            for c in range(NCH):
                nc.scalar.dma_start(
                    out=_ap(out3_o, 2048 * c, [[128, 16], [BC, 3], [1, 128]]),
                    in_=outsb[16 * c:16 * c + 16, :].rearrange("p (m q) -> p m q", m=3),
                )
    return nc


def _get_nc():
    if "nc" not in _CACHE:
        nc = bacc.Bacc("TRN2", target_bir_lowering=False, debug=False)
        _build_program(nc)
        nc.compile()
        _CACHE["nc"] = nc
    return _CACHE["nc"]


def _host_consts(W1, b1, W2, b2, Wh, bh, Wc1, bc1, Wc2, bc2, Wc3, bc3):
    import ml_dtypes
    W1cat = np.concatenate([W1, Wc1], axis=1).astype(ml_dtypes.bfloat16).astype(np.float32)
    # K split into two full-128 chunks: features 0:128 and 66:194 (transpose
    # windows); overlapped features 66:128 contribute half from each chunk.
    W1cat_a = W1cat[0:128].copy()
    W1cat_a[66:128] *= 0.5
    W1cat_b = W1cat[66:194].copy()
    W1cat_b[0:62] *= 0.5
    W1cat_a = W1cat_a.astype(ml_dtypes.bfloat16)
    W1cat_b = W1cat_b.astype(ml_dtypes.bfloat16)
    b1cat = np.concatenate([b1, bc1]).astype(np.float32).reshape(128, 1)
    W2cat = np.zeros((128, 128), np.float32)
    W2cat[:64, :64] = W2
    W2cat[64:, 64:] = Wc2
    b2cat = np.concatenate([b2, bc2]).astype(np.float32).reshape(128, 1)
    W3cat = np.zeros((128, 32), np.float32)
    W3cat[:64, :15] = np.asarray(Wh).transpose(1, 0, 2).reshape(64, 15)
    wc3 = np.asarray(Wc3)[:, 0].astype(np.float32)
    wc3_hi = wc3.astype(ml_dtypes.bfloat16).astype(np.float32)
    W3cat[64:, 15] = wc3_hi
    W3cat[64:, 16] = wc3 - wc3_hi
    W3cat = W3cat.astype(ml_dtypes.bfloat16)
    b3cat = np.zeros(32, np.float32)
    b3cat[:15] = np.asarray(bh).reshape(15)
    b3cat[15] = np.asarray(bc3)[0]
    b512 = np.tile(np.tile(b3cat, 16)[None, :], (128, 1)).astype(np.float32)
    idm = np.eye(128, dtype=np.float32)
    iota = np.tile(np.arange(5, dtype=np.float32), (128, 1))
    return dict(w1cat=np.ascontiguousarray(W1cat_a), w1catb=np.ascontiguousarray(W1cat_b), w2cat=W2cat, w3cat=W3cat,
                b1cat=b1cat, b2cat=b2cat, b512=b512, idm=idm, iota=iota)


def kernel(x, action, W1, b1, W2, b2, Wh, bh, Wc1, bc1, Wc2, bc2, Wc3, bc3):
    global LAST_RESULT
    x = np.ascontiguousarray(np.asarray(x, dtype=np.float32))
    action = np.asarray(action)
    consts = _host_consts(W1, b1, W2, b2, Wh, bh, Wc1, bc1, Wc2, bc2, Wc3, bc3)
    nc = _get_nc()
    act_f32 = np.ascontiguousarray(action.astype(np.float32))
    in_maps = []
    for c in range(NCORES):
        m = {"xg": x[c * BC:(c + 1) * BC], "act": act_f32[c * BC:(c + 1) * BC]}
        m.update(consts)
        in_maps.append(m)
    res = run_bass_kernel_spmd(nc, in_maps, list(range(NCORES)))
    LAST_RESULT = res
    logp = np.concatenate([res.results[c]["out3_o"][0] for c in range(NCORES)])
    ent = np.concatenate([res.results[c]["out3_o"][1] for c in range(NCORES)])
    val = np.concatenate([res.results[c]["out3_o"][2] for c in range(NCORES)])
    return action, logp, ent, val[:, None]


# revision 26
# speedup vs baseline: 1.0625x; 1.0625x over previous
"""Trainium2 Bass kernel for nn_Agent_12489764897159 (moe_routing actor-critic MLP).

Data-parallel over 8 NeuronCores: batch B=131072 split into 8 x 16384.
All weights replicated (fused/padded host-side into 3 matmul operands).

Per-core device pipeline (feature-major trunk, batch-major postprocessing):
  x [16384,194] --DMA--> SBUF batch-major --PE transpose--> xT feature-major
  out1 = W1cat.T @ xT          (fp32r matmuls, actor+critic trunks fused: 64+64 rows)
  h1   = tanh(out1 + b1cat)    (ACT, PSUM->SBUF)
  out2 = W2cat.T @ h1; h2 = tanh(out2 + b2cat)
  o3   = W3cat.T @ h2          (col-tiled 4x32: [all_logits(15) | value | pad] stacked x4)
  o3s  = o3 + bias3            (DVE, PSUM->SBUF)
  o3bm = PE-transpose(o3s)     (batch-major: samples on partitions)
  POST accumulates o3bm; postprocessing (event-mask select, softmax stats,
  entropy, logp gather by action) runs once per core on [128, 4096] views.
"""

import numpy as np
import concourse.bacc as bacc
import concourse.bass as bass
import concourse.mybir as mybir
from concourse import tile
from concourse.bass_utils import run_bass_kernel_spmd

F32 = mybir.dt.float32
F32R = mybir.dt.float32r
BF16 = mybir.dt.bfloat16
I32 = mybir.dt.int32
AF = mybir.ActivationFunctionType
OP = mybir.AluOpType

NCORES = 8
B, OBS, H, A, E = 131072, 194, 64, 5, 3
BC = B // NCORES          # 16384 samples per core
NCH = 8                   # chunks per core
CH = 2048                 # samples per chunk
G_PER_CHUNK = 16          # 128-sample groups per chunk
NG = NCH * G_PER_CHUNK    # 128 groups per core

_CACHE = {}
LAST_RESULT = None


def _ap(t, offset, dims):
    return bass.AP(t.tensor if isinstance(t, bass.AP) else t, offset, dims)


def _build_program(nc):
    xg = nc.dram_tensor("xg", [BC, OBS], F32, kind="ExternalInput")
    act = nc.dram_tensor("act", [BC], F32, kind="ExternalInput")
    w1 = nc.dram_tensor("w1cat", [128, 128], BF16, kind="ExternalInput")
    w1b = nc.dram_tensor("w1catb", [128, 128], BF16, kind="ExternalInput")
    w2 = nc.dram_tensor("w2cat", [128, 128], F32R, kind="ExternalInput")
    w3 = nc.dram_tensor("w3cat", [128, 32], BF16, kind="ExternalInput")
    b1 = nc.dram_tensor("b1cat", [128, 1], F32, kind="ExternalInput")
    b2 = nc.dram_tensor("b2cat", [128, 1], F32, kind="ExternalInput")
    b512 = nc.dram_tensor("b512", [128, 512], F32, kind="ExternalInput")
    idm = nc.dram_tensor("idm", [128, 128], F32, kind="ExternalInput")
    iota = nc.dram_tensor("iota", [128, 5], F32, kind="ExternalInput")
    out3_o = nc.dram_tensor("out3_o", [3, BC], F32, kind="ExternalOutput")

    with tile.TileContext(nc) as tc:
        with (
            tc.tile_pool(name="constp", bufs=1) as constp,
            tc.tile_pool(name="xrp", bufs=3) as xrp,
            tc.tile_pool(name="xbmp", bufs=3) as xbmp,
            tc.tile_pool(name="xtsp", bufs=3) as xtsp,
            tc.tile_pool(name="hp", bufs=3) as hp,
            tc.tile_pool(name="bigp", bufs=1) as bigp,
            tc.tile_pool(name="tpp", bufs=1, space="PSUM") as tpp,
            tc.tile_pool(name="mmp", bufs=5, space="PSUM") as mmp,
            tc.tile_pool(name="o3bmp", bufs=2, space="PSUM") as o3bmp,
        ):
            # ---- constants ----
            w1a_sb = constp.tile([128, 128], BF16)
            nc.scalar.dma_start(out=w1a_sb[:], in_=w1.ap())
            w1b_sb = constp.tile([128, 128], BF16)
            nc.scalar.dma_start(out=w1b_sb[:], in_=w1b.ap())
            w2_sb = constp.tile([128, 128], F32R)
            nc.scalar.dma_start(out=w2_sb[:], in_=w2.ap())
            w3_sb = constp.tile([128, 32], BF16)
            nc.scalar.dma_start(out=w3_sb[:], in_=w3.ap())
            b1_sb = constp.tile([128, 1], F32)
            nc.scalar.dma_start(out=b1_sb[:], in_=b1.ap())
            b2_sb = constp.tile([128, 1], F32)
            nc.scalar.dma_start(out=b2_sb[:], in_=b2.ap())
            b512_sb = constp.tile([128, 512], F32)
            nc.scalar.dma_start(out=b512_sb[:], in_=b512.ap())
            idm_sb = constp.tile([128, 128], F32)
            nc.scalar.dma_start(out=idm_sb[:], in_=idm.ap())
            iota_sb = constp.tile([128, 5], F32)
            nc.scalar.dma_start(out=iota_sb[:], in_=iota.ap())

            # ---- persistent per-core tiles ----
            POST = bigp.tile([128, 32 * NG], F32)     # col = 512c+128t+32j+m
            x012 = bigp.tile([128, 3 * NG], F32)      # col = 3G+i, G=16c+4t+j
            actf = bigp.tile([128, NG], F32)

            # ---- action: dense k-major load, PE transpose to batch-major ----
            # host supplies act as fp32; act[k, p] with sample = 128k + p
            actk = bigp.tile([128, 128], F32)
            nc.scalar.dma_start(out=actk[:], in_=_ap(act, 0, [[128, 128], [1, 128]]))
            actp = tpp.tile([128, 512], F32, tag="tp", name="actp")
            nc.tensor.transpose(actp[:, 0:128], actk[:], idm_sb[:])
            # psum[p, k] -> actf[p, G] with G = 16c+4t+j, k = 16c+4j+t
            nc.vector.tensor_copy(
                _ap(actf, 0, [actf.ap[0], [16, 8], [4, 4], [1, 4]]),
                _ap(actp, actp.offset, [actp.ap[0], [16, 8], [1, 4], [4, 4]]))

            # ---- main per-chunk pipeline (software-pipelined emission) ----
            state = {}

            def emit_input(c):
                # fp32 batch-major load, large descriptors (194-elem runs)
                x_bm = xbmp.tile([128, 16 * OBS], F32)
                nc.gpsimd.dma_start(
                    out=_ap(x_bm, 0, [x_bm.ap[0], [OBS, 16], [1, OBS]]),
                    in_=_ap(xg, c * CH * OBS, [[OBS, 128], [128 * OBS, 16], [1, OBS]]),
                )
                # extract x[:, 0:3] (fp32) for the event masks: blk = 4j + t
                nc.vector.tensor_copy(
                    _ap(x012, 48 * c, [x012.ap[0], [3, 4], [12, 4], [1, 3]]),
                    _ap(x_bm, 0, [x_bm.ap[0], [4 * OBS, 4], [OBS, 4], [1, 3]]))
                # cast+reshuffle f32 -> bf16 xbar layout:
                # R1 cols 0:2048 = (blk, f 0:128); R2 = (blk, 66+g) overlap window
                xr = xrp.tile([128, 4096], BF16)
                nc.vector.tensor_copy(
                    _ap(xr, 0, [xr.ap[0], [128, 16], [1, 128]]),
                    _ap(x_bm, 0, [x_bm.ap[0], [OBS, 16], [1, 128]]))
                nc.vector.tensor_copy(
                    _ap(xr, 2048, [xr.ap[0], [128, 16], [1, 128]]),
                    _ap(x_bm, 66, [x_bm.ap[0], [OBS, 16], [1, 128]]))
                xT = xtsp.tile([128, 2 * CH], BF16, tag="xT")
                nc.sync.dma_start_transpose(
                    xT[:, 0:CH].rearrange("f (b s) -> f b s", s=128), xr[:, 0:2048])
                nc.scalar.dma_start_transpose(
                    xT[:, CH:2 * CH].rearrange("f (b s) -> f b s", s=128),
                    xr[:, 2048:4096])

                state[c] = xT

            def emit_compute(c):
                xT = state.pop(c)
                h1 = hp.tile([128, CH], F32R, tag="h", name=f"h1_{c}")
                h2 = hp.tile([128, CH], BF16, tag="h2", name=f"h2_{c}")
                o12s = [mmp.tile([128, 512], F32, tag="mm", name=f"o1_{c}_{j}")
                        for j in range(4)]
                for j in range(4):
                    nc.tensor.matmul(
                        o12s[j][:], w1a_sb[:],
                        xT[:, 512 * j:512 * j + 512],
                        start=True, stop=False)
                for j in range(4):
                    nc.tensor.matmul(
                        o12s[j][:], w1b_sb[:],
                        xT[:, CH + 512 * j:CH + 512 * j + 512],
                        start=False, stop=True)
                for j in range(4):
                    nc.scalar.activation(
                        out=h1[:, 512 * j:512 * j + 512], in_=o12s[j][:],
                        func=AF.Tanh, bias=b1_sb[:], scale=1.0)
                for j in range(4):
                    o12 = mmp.tile([128, 512], F32, tag="mm", name=f"o2_{c}_{j}")
                    nc.tensor.matmul(
                        o12[:], w2_sb[:],
                        h1[:, 512 * j:512 * j + 512])
                    nc.scalar.activation(
                        out=h2[:, 512 * j:512 * j + 512], in_=o12[:],
                        func=AF.Tanh, bias=b2_sb[:], scale=1.0)

                # batch-stationary head matmuls: lhsT = h2 block ->
                # out [128 samples, 32 metrics] batch-major directly
                o3bm = o3bmp.tile([128, 512], F32, tag="o3bm", name=f"o3bm_{c}")
                for j in range(4):
                    for t in range(4):
                        nc.tensor.matmul(
                            o3bm[:, 32 * (4 * t + j):32 * (4 * t + j) + 32],
                            h2[:, 512 * j + 128 * t:512 * j + 128 * t + 128],
                            w3_sb[:])
                # POST accumulate with bias pattern (bh | bc3) folded in
                nc.vector.tensor_tensor(
                    POST[:, 512 * c:512 * c + 512], o3bm[:], b512_sb[:], op=OP.add)

            LA = 2
            for c in range(NCH + LA):
                if c < NCH:
                    emit_input(c)
                if c >= LA:
                    emit_compute(c - LA)

            # ---- per-core postprocessing (batch-major, G=128 groups) ----
            P3 = POST.rearrange("p (g m) -> p g m", m=32)      # [128,128,32]
            X3 = x012.rearrange("p (g i) -> p g i", i=3)

            ge01 = bigp.tile([128, NG], F32)
            ge02 = bigp.tile([128, NG], F32)
            ge12 = bigp.tile([128, NG], F32)
            nc.vector.tensor_tensor(ge01[:], X3[:, :, 0], X3[:, :, 1], op=OP.is_ge)
            nc.vector.tensor_tensor(ge02[:], X3[:, :, 0], X3[:, :, 2], op=OP.is_ge)
            nc.vector.tensor_tensor(ge12[:], X3[:, :, 1], X3[:, :, 2], op=OP.is_ge)
            m0 = bigp.tile([128, NG], F32)
            nc.vector.tensor_mul(m0[:], ge01[:], ge02[:])
            m1t = bigp.tile([128, NG], F32)
            nc.vector.tensor_mul(m1t[:], ge01[:], ge12[:])
            m1 = bigp.tile([128, NG], F32)
            nc.vector.tensor_sub(m1[:], ge12[:], m1t[:])

            L0 = P3[:, :, 0:5]
            L1 = P3[:, :, 5:10]
            L2 = P3[:, :, 10:15]
            D0 = bigp.tile([128, 5 * NG], F32)
            D1 = bigp.tile([128, 5 * NG], F32)
            D0r = D0.rearrange("p (g a) -> p g a", a=5)
            D1r = D1.rearrange("p (g a) -> p g a", a=5)
            nc.vector.tensor_sub(D0r, L0, L2)
            nc.vector.tensor_sub(D1r, L1, L2)

            def bcast5(m):
                return _ap(m, m.offset, [m.ap[0], m.ap[1], [0, 5]])

            SEL = bigp.tile([128, 5 * NG], F32)
            SELr = SEL.rearrange("p (g a) -> p g a", a=5)
            # SEL = L2 + m0*D0 + m1*D1
            nc.vector.tensor_mul(D0r, D0r, bcast5(m0))
            nc.vector.tensor_mul(D1r, D1r, bcast5(m1))
            nc.vector.tensor_add(SELr, L2, D0r)
            nc.vector.tensor_add(SELr, SELr, D1r)

            OH = bigp.tile([128, 5 * NG], F32)
            OHr = OH.rearrange("p (g a) -> p g a", a=5)
            actb = _ap(actf, actf.offset, [actf.ap[0], actf.ap[1], [0, 5]])
            iotab = _ap(iota_sb, iota_sb.offset, [iota_sb.ap[0], [0, NG], [1, 5]])
            nc.vector.tensor_tensor(OHr, actb, iotab, op=OP.is_equal)
            LSW = bigp.tile([128, 5 * NG], F32)
            LSWr = LSW.rearrange("p (g a) -> p g a", a=5)
            nc.vector.tensor_mul(LSWr, SELr, OHr)
            LSEL = bigp.tile([128, NG], F32)
            nc.vector.tensor_reduce(LSEL[:], LSWr, axis=mybir.AxisListType.X, op=OP.add)

            EXP = bigp.tile([128, 5 * NG], F32)
            EXPr = EXP.rearrange("p (g a) -> p g a", a=5)
            nc.scalar.activation(out=EXP[:], in_=SEL[:], func=AF.Exp)
            S = bigp.tile([128, NG], F32)
            nc.vector.tensor_reduce(S[:], EXPr, axis=mybir.AxisListType.X, op=OP.add)
            TW = bigp.tile([128, 5 * NG], F32)
            TWr = TW.rearrange("p (g a) -> p g a", a=5)
            nc.vector.tensor_mul(TWr, SELr, EXPr)
            T = bigp.tile([128, NG], F32)
            nc.vector.tensor_reduce(T[:], TWr, axis=mybir.AxisListType.X, op=OP.add)
            logS = bigp.tile([128, NG], F32)
            nc.scalar.activation(out=logS[:], in_=S[:], func=AF.Ln)
            Sinv = bigp.tile([128, NG], F32)
            nc.vector.reciprocal(Sinv[:], S[:])
            TSi = bigp.tile([128, NG], F32)
            nc.vector.tensor_mul(TSi[:], T[:], Sinv[:])

            # outputs staged k-ordered (k = 16c + 4j + t) for contiguous stores
            outst = bigp.tile([128, 384], F32)

            def kview(mi):
                # write view enumerating (c, t, j): col = 128*mi + 16c + 4j + t
                return _ap(outst, outst.offset + 128 * mi,
                           [outst.ap[0], [16, 8], [1, 4], [4, 4]])

            def gview(tl):
                return _ap(tl, tl.offset, [tl.ap[0], [16, 8], [4, 4], [1, 4]])

            nc.vector.tensor_tensor(kview(0), gview(LSEL), gview(logS), op=OP.subtract)
            nc.vector.tensor_tensor(kview(1), gview(logS), gview(TSi), op=OP.subtract)
            nc.vector.tensor_tensor(
                kview(2),
                _ap(POST, POST.offset + 15, [POST.ap[0], [512, 8], [128, 4], [32, 4]]),
                _ap(POST, POST.offset + 16, [POST.ap[0], [512, 8], [128, 4], [32, 4]]),
                op=OP.add)

            otp = tpp.tile([128, 512], F32, tag="tp")
            for mi in range(3):
                nc.tensor.transpose(
                    otp[:, 128 * mi:128 * mi + 128],
                    outst[:, 128 * mi:128 * mi + 128], idm_sb[:])
            outsb = bigp.tile([128, 384], F32)
            nc.vector.tensor_copy(outsb[:], otp[:, 0:384])

            for c in range(NCH):
                nc.scalar.dma_start(
                    out=_ap(out3_o, 2048 * c, [[128, 16], [BC, 3], [1, 128]]),
                    in_=outsb[16 * c:16 * c + 16, :].rearrange("p (m q) -> p m q", m=3),
                )
    return nc


def _get_nc():
    if "nc" not in _CACHE:
        nc = bacc.Bacc("TRN2", target_bir_lowering=False, debug=False)
        _build_program(nc)
        nc.compile()
        _CACHE["nc"] = nc
    return _CACHE["nc"]


def _host_consts(W1, b1, W2, b2, Wh, bh, Wc1, bc1, Wc2, bc2, Wc3, bc3):
    import ml_dtypes
    W1cat = np.concatenate([W1, Wc1], axis=1).astype(ml_dtypes.bfloat16).astype(np.float32)
    # K split into two full-128 chunks: features 0:128 and 66:194 (transpose
    # windows); overlapped features 66:128 contribute half from each chunk.
    W1cat_a = W1cat[0:128].copy()
    W1cat_a[66:128] *= 0.5
    W1cat_b = W1cat[66:194].copy()
    W1cat_b[0:62] *= 0.5
    W1cat_a = W1cat_a.astype(ml_dtypes.bfloat16)
    W1cat_b = W1cat_b.astype(ml_dtypes.bfloat16)
    b1cat = np.concatenate([b1, bc1]).astype(np.float32).reshape(128, 1)
    W2cat = np.zeros((128, 128), np.float32)
    W2cat[:64, :64] = W2
    W2cat[64:, 64:] = Wc2
    b2cat = np.concatenate([b2, bc2]).astype(np.float32).reshape(128, 1)
    W3cat = np.zeros((128, 32), np.float32)
    W3cat[:64, :15] = np.asarray(Wh).transpose(1, 0, 2).reshape(64, 15)
    wc3 = np.asarray(Wc3)[:, 0].astype(np.float32)
    wc3_hi = wc3.astype(ml_dtypes.bfloat16).astype(np.float32)
    W3cat[64:, 15] = wc3_hi
    W3cat[64:, 16] = wc3 - wc3_hi
    W3cat = W3cat.astype(ml_dtypes.bfloat16)
    b3cat = np.zeros(32, np.float32)
    b3cat[:15] = np.asarray(bh).reshape(15)
    b3cat[15] = np.asarray(bc3)[0]
    b512 = np.tile(np.tile(b3cat, 16)[None, :], (128, 1)).astype(np.float32)
    idm = np.eye(128, dtype=np.float32)
    iota = np.tile(np.arange(5, dtype=np.float32), (128, 1))
    return dict(w1cat=np.ascontiguousarray(W1cat_a), w1catb=np.ascontiguousarray(W1cat_b), w2cat=W2cat, w3cat=W3cat,
                b1cat=b1cat, b2cat=b2cat, b512=b512, idm=idm, iota=iota)


def kernel(x, action, W1, b1, W2, b2, Wh, bh, Wc1, bc1, Wc2, bc2, Wc3, bc3):
    global LAST_RESULT
    x = np.ascontiguousarray(np.asarray(x, dtype=np.float32))
    action = np.asarray(action)
    consts = _host_consts(W1, b1, W2, b2, Wh, bh, Wc1, bc1, Wc2, bc2, Wc3, bc3)
    nc = _get_nc()
    act_f32 = np.ascontiguousarray(action.astype(np.float32))
    in_maps = []
    for c in range(NCORES):
        m = {"xg": x[c * BC:(c + 1) * BC], "act": act_f32[c * BC:(c + 1) * BC]}
        m.update(consts)
        in_maps.append(m)
    res = run_bass_kernel_spmd(nc, in_maps, list(range(NCORES)))
    LAST_RESULT = res
    logp = np.concatenate([res.results[c]["out3_o"][0] for c in range(NCORES)])
    ent = np.concatenate([res.results[c]["out3_o"][1] for c in range(NCORES)])
    val = np.concatenate([res.results[c]["out3_o"][2] for c in range(NCORES)])
    return action, logp, ent, val[:, None]
